# revision 4
# baseline (speedup 1.0000x reference)
"""Trainium2 Bass kernel for nn_DiffPhysKAN.

Reaction-diffusion PDE (SIR-like) explicitly time-stepped T=100 times over a
1D grid of N=500000 points, with per-step beta(t) from a tiny KAN network and
a learned diffusion coefficient.

Strategy (v2 — u32 fixed-point state):
  - beta(t)/diff/dt/dx are tiny host-side scalar computations (T=100 values);
    they are baked into the device program as per-step immediates.
  - The spatial grid is sharded over 8 NeuronCores (1D domain decomposition).
    The replicate-boundary stencil is exactly a mirror (Neumann) boundary, so
    the host mirror-pads the initial condition; each core gets its 62500-col
    chunk plus 110-element halos and runs all 100 steps with ZERO collectives
    (ghost-zone trick: errors from stale halos advance 1 element/step and
    never reach the output region).
  - The state is kept in uint32 fixed point, J = I * (2^32-1)/10, so that the
    DVE's saturating f32->u32 write conversion performs clip(I,0,10) for
    free: J'=0 at I=0 and J'=2^32-1 at I=10 exactly. One custom 8-block DVE
    op per step computes
        S = a*(L + R) + M*(c1 - b*M);  relu;  saturating u32 round
    (a = dt*diff/dx^2 in shared I/J units, b = dt*beta_t/S32,
    c1 = 1 - 2a - dt + dt*beta_t), with the left tap L synthesized from the
    center stream by the swap-flop delay trick. u32 quantization (~5e-6 per
    step after f32 rounding) tracks the f32 reference as well as a pure-f32
    kernel does (sim: rel err 1.6e-6).
  - The ACT (scalar) engine, otherwise idle, converts each new state's 490
    data cols to u16 history (x 65535/(2^32-1)) into a persistent SBUF tile
    [128, T*W]; the Sync engine ships it to DRAM in multi-step contiguous
    chunks (~1.4 MB per dma_start, one fat descriptor per partition).
  - Partition-level ghosts are refreshed every 20 steps by two SBUF->SBUF
    DMAs shifted by one partition (staged 4 steps early), installed with two
    cheap same-engine DVE copies so the DVE never waits on a DMA.
"""

import sys

for _p in ("/opt/trn_rl_repo", "/root/.axon_site/_ro/trn_rl_repo"):
    if _p not in sys.path:
        sys.path.append(_p)

import numpy as np

f32 = np.float32
f64 = np.float64

# ---- problem/layout constants (hardcoded per contest contract) ----
T = 100
N = 500000
NCORES = 8
OUT = N // NCORES        # 62500 output cols per core
P = 128                  # SBUF partitions
C = 490                  # data cols per partition (128*490 = 62720 per core)
CORE_SLICE = P * C       # 62720
HALO = (CORE_SLICE - OUT) // 2   # 110 (>= T=100 needed)
DL = 28                  # left ghost cols
DR = 28                  # right ghost cols (W even -> 4B-aligned u16 rows)
W = DL + C + DR          # 546
PAD_L = HALO + DL        # host mirror-pad widths
PAD_R = HALO + DR
REFRESH_EVERY = 20       # ghost refresh period (staleness 4 + fronts < DL/DR)

UMAX = 4294967295.0
S32 = UMAX / 10.0                  # J = I * S32 (f64 scale on host)
C16 = float(np.float32(65535.0 / UMAX))   # u16 out = sat_round(f32(J) * C16)
S16 = 6553.5                       # I = u16 / S16

# DMA chunk sizes (steps per hist dma_start), tapered so the final chunks
# expose minimal tail latency after the last compute step. History stages
# through a rolling 2-chunk parity buffer in SBUF so the ACT writes of chunk
# k+1 never wait on the DMA read of chunk k.
CHUNKS = [12, 12, 12, 12, 12, 12, 12, 8, 4, 2, 1, 1]
CHMAX = max(CHUNKS)
assert sum(CHUNKS) == T

# ---------------------------------------------------------------- host math


def _softplus(x):
    x = x.astype(f32)
    return (np.maximum(x, 0) + np.log1p(np.exp(-np.abs(x), dtype=f32), dtype=f32)).astype(f32)


def _kan_layer(x, grid, spline_w, base_w):
    x = x.astype(f32)
    base = x @ base_w.T.astype(f32)
    basis = np.exp(-((x[:, :, None] - grid[None, None, :]) ** 2) * f32(10.0), dtype=f32)
    basis = basis.reshape(x.shape[0], -1)
    return (base + basis @ spline_w).astype(f32)


def _host_params(t_steps, x_grid, grid1, spline_w1, base_w1, grid2, spline_w2,
                 base_w2, diff_param):
    h = _kan_layer(t_steps, grid1, spline_w1, base_w1)
    h = _kan_layer(h, grid2, spline_w2, base_w2)
    betas = np.clip(_softplus(h), 0.0, 20.0).astype(f32).reshape(-1)
    diff = np.clip(_softplus(diff_param), 0.0, 1.0).astype(f32)[0]
    dt = f32(t_steps[1, 0] - t_steps[0, 0])
    dx = f32(x_grid[1] - x_grid[0])
    a = f32(np.float64(dt) * np.float64(diff) / (np.float64(dx) ** 2))
    b_all = [f32(np.float64(dt) * np.float64(b)) for b in betas]
    c1_all = [f32(1.0 - 2 * np.float64(a) - np.float64(dt) + np.float64(b)) for b in b_all]
    return a, b_all, c1_all


# ------------------------------------------------------- custom DVE op

_OPS_CACHE = {}


def _get_custom_ops():
    """Register PDE_FUSED_S: a hand-written 8-block DVE micro-op computing
        S[e] = relu(a*(L + R) + M*(c1 - b*M))
    in ONE pass, where M = in0 (center view), R = in1 (right view) and the
    left tap L = M delayed by one element, synthesized with the swap flop
    (block0 BYPASS latches operand B; CURR_SWAP_OUT reads the previous
    element's value). Consts: C0=b (s0), C1=c1 (s1), C2=a (imm2).
    out[0] is garbage (uninitialized swap flop) — it lands in a ghost
    column and never reaches the output region. With a uint32 output AP the
    write conversion saturates at [0, 2^32-1], providing the upper clip."""
    if _OPS_CACHE:
        return _OPS_CACHE["S"]
    import concourse.dve_ops as D
    from concourse.dve_spec import Spec, Src0, Src1, C0, C1, C2
    from concourse.dve_uop import (UopConfig, DveOpSpec, InpSel, AluInp, AluOp,
                                   OutSel, OutPath, Trigger, DelayInp)
    ENABLE = 1

    name = "PDE_FUSED_S"
    for op in D.OPS:
        if op.name == name:
            _OPS_CACHE["S"] = op
            return op

    u = UopConfig()
    u.enable_input(InpSel.SRC_0, 1)      # M-view   -> chain0 feed
    u.enable_input(InpSel.SRC_1, 2)      # R-view   -> chain1 feed
    u.enable_input(InpSel.CONST_0, 3)    # b        -> chain2 feed
    u.enable_input(InpSel.CONST_1, 4)    # c1       -> chain3 feed
    u.enable_input(InpSel.CONST_2, 5)    # a        -> chain4 feed
    u.enable_input(InpSel.ZERO, 6)       # 0        -> chain5 feed
    u.require_inp0 = ENABLE
    u.require_inp1 = ENABLE
    u.trigger = (Trigger.SRC_TENSOR_DONE, Trigger.NONE, Trigger.NONE)
    dp = u.datapath_config
    # b0: L = delayed M  (BYPASS passes A=CURR_SWAP_OUT; swap latches B=M)
    dp[0].enable_alu(AluOp.BYPASS, AluInp.CURR_SWAP_OUT, AluInp.PREV_DELAY_0)
    dp[0].swap_enable = ENABLE
    dp[0].pass_through_delay(0, 1, 2, 3, 4, 5)
    # b1: u = L + R
    dp[1].enable_alu(AluOp.ADD, AluInp.PREV_ALU_OUT, AluInp.PREV_DELAY_1)
    dp[1].pass_through_delay(0, 2, 3, 4, 5)
    # b2: t1 = M * b ; park u in chain1
    dp[2].enable_alu(AluOp.MULTIPLY, AluInp.PREV_DELAY_0, AluInp.PREV_DELAY_2)
    dp[2].enable_delay_from_src(DelayInp.PREV_ALU_OUT, 1)
    dp[2].pass_through_delay(0, 3, 4, 5)
    # b3: t2 = c1 - t1
    dp[3].enable_alu(AluOp.SUBTRACT, AluInp.PREV_DELAY_3, AluInp.PREV_ALU_OUT)
    dp[3].pass_through_delay(0, 1, 4, 5)
    # b4: Q = t2 * M
    dp[4].enable_alu(AluOp.MULTIPLY, AluInp.PREV_ALU_OUT, AluInp.PREV_DELAY_0)
    dp[4].pass_through_delay(1, 4, 5)
    # b5: au = u * a ; park Q in chain0
    dp[5].enable_alu(AluOp.MULTIPLY, AluInp.PREV_DELAY_1, AluInp.PREV_DELAY_4)
    dp[5].enable_delay_from_src(DelayInp.PREV_ALU_OUT, 0)
    dp[5].pass_through_delay(5)
    # b6: S = au + Q
    dp[6].enable_alu(AluOp.ADD, AluInp.PREV_ALU_OUT, AluInp.PREV_DELAY_0)
    dp[6].pass_through_delay(5)
    # b7: max(S, 0) — lower clip (redundant with u32 saturation, kept)
    dp[7].enable_alu(AluOp.MAX, AluInp.PREV_ALU_OUT, AluInp.PREV_DELAY_5)
    u.enable_output(OutSel.ALU_OUT, OutPath.WR0_LO)

    def _ref(in0, in1, s0, s1, imm2):
        in0 = in0.astype(np.float32)
        in1 = in1.astype(np.float32)
        L = np.concatenate([in0[:, :1], in0[:, :-1]], axis=1)
        return np.maximum(
            imm2 * (L + in1) + in0 * (s1 - in0 * s0), 0.0).astype(np.float32)

    spec = Spec(body=(Src0 + Src1) * C2 + Src0 * (C1 - Src0 * C0),
                reference=_ref)
    op = D.DveOp(name, spec, subdim=False, uops_sha={})
    D.OPS.append(op)
    D._SUB_OPCODE_FOR_NAME[name] = D._CUSTOM_DVE_ROW_BASE + len(D.OPS) - 1
    D.CUSTOM_DVE_SPECS[name] = spec
    opspec = DveOpSpec(name=name, opcode=D._SUB_OPCODE_FOR_NAME[name],
                       uops=[u], rd1_en=True)
    for ver in ("v3", "v4"):
        D._COMPILE_CACHE[(name, ver)] = opspec
    _OPS_CACHE["S"] = op
    return op


# ------------------------------------------------------- device program


def _build_program(a, b_all, c1_all):
    from concourse import bacc, mybir
    from concourse.tile import TileContext

    op_s = _get_custom_ops()
    nc = bacc.Bacc(None, target_bir_lowering=False)
    x0 = nc.declare_dram_parameter("x0", [P, W], mybir.dt.uint32, isOutput=False)
    hist = nc.declare_dram_parameter("hist", [P, T * W], mybir.dt.uint16,
                                     isOutput=True)

    # per-step J-unit constants: b_t scaled into u32 units, c1 unchanged
    b32_all = [float(np.float32(np.float64(b) / S32)) for b in b_all]
    c1f_all = [float(c) for c in c1_all]
    af = float(a)

    with TileContext(nc) as tc:
        with tc.tile_pool(name="x", bufs=6) as xpool, \
             tc.tile_pool(name="h", bufs=1) as hpool, \
             tc.tile_pool(name="g", bufs=2) as gpool:
            H = hpool.tile([P, 2 * CHMAX * W], mybir.dt.uint16)
            X = xpool.tile([P, W], mybir.dt.uint32)
            nc.sync.dma_start(out=X[:, :], in_=x0[:, :])
            pending = None
            done = 0
            nxt = 0
            par = 0
            for t in range(T):
                Xn = xpool.tile([P, W], mybir.dt.uint32)
                nc.vector._custom_dve(op_s, out=Xn[:, 2:W - 1],
                                      in0=X[:, 2:W - 1], in1=X[:, 3:W],
                                      s0=b32_all[t], s1=c1f_all[t],
                                      imm2=af)
                # ACT: u32 state -> u16 history (saturating round on write)
                hoff = (par * CHMAX + (t - done)) * W
                nc.scalar.mul(H[:, hoff + DL:hoff + DL + C],
                              Xn[:, DL:DL + C], C16)
                X = Xn
                # Ghost refresh: stage partition-shifted halo data via the idle
                # GpSimd (SWDGE) queue four steps early (even staleness matches
                # the saturated field's period-2 oscillation; garbage fronts
                # stay below DL/DR), then install with two cheap same-engine
                # DVE copies so the DVE never waits on a DMA.
                if (t + 5) % REFRESH_EVERY == 0 and (t + 5) < T:
                    gl = gpool.tile([P, DL], mybir.dt.uint32, tag="gl")
                    gr = gpool.tile([P, DR], mybir.dt.uint32, tag="gr")
                    nc.gpsimd.dma_start(out=gl[1:P, :], in_=X[0:P - 1, C:C + DL])
                    nc.gpsimd.dma_start(out=gr[0:P - 1, :], in_=X[1:P, DL:DL + DR])
                    pending = (gl, gr)
                if (t + 1) % REFRESH_EVERY == 0 and (t + 1) < T:
                    gl, gr = pending
                    nc.vector.tensor_copy(X[:, 0:DL], gl[:, :])
                    nc.vector.tensor_copy(X[:, C + DL:W], gr[:, :])
                # chunked history DMA (contiguous per partition), parity-
                # alternating source so ACT never waits on the DMA drain
                if done + CHUNKS[nxt] == t + 1:
                    k = CHUNKS[nxt]
                    lo = par * CHMAX * W
                    nc.sync.dma_start(out=hist[:, done * W:(t + 1) * W],
                                      in_=H[:, lo:lo + k * W])
                    done = t + 1
                    nxt += 1
                    par ^= 1
    nc.finalize()
    return nc


# ------------------------------------------------------------- entry points


def _run(inputs, trace=False, trace_kwargs=None):
    from concourse.bass_utils import run_bass_kernel_spmd

    t_steps = np.asarray(inputs["t_steps"], f32)
    x_grid = np.asarray(inputs["x_grid"], f32)
    initial_I = np.asarray(inputs["initial_I"], f32)
    a, b_all, c1_all = _host_params(
        t_steps, x_grid,
        np.asarray(inputs["grid1"], f32), np.asarray(inputs["spline_w1"], f32),
        np.asarray(inputs["base_w1"], f32),
        np.asarray(inputs["grid2"], f32), np.asarray(inputs["spline_w2"], f32),
        np.asarray(inputs["base_w2"], f32), np.asarray(inputs["diff_param"], f32))

    G = np.pad(initial_I, (PAD_L, PAD_R), mode="symmetric")
    J = np.rint(G.astype(f64) * S32).astype(np.uint32)
    sw = np.lib.stride_tricks.sliding_window_view(J, W)
    row0 = np.arange(P) * C
    in_maps = []
    for c in range(NCORES):
        tile = np.ascontiguousarray(sw[c * OUT + row0], dtype=np.uint32)
        in_maps.append({"x0": tile})

    nc = _build_program(a, b_all, c1_all)
    res = run_bass_kernel_spmd(nc, in_maps, core_ids=list(range(NCORES)),
                               trace=trace, trace_kwargs=trace_kwargs or {})

    out = np.empty((T, N), f32)
    inv = f32(1.0) / f32(S16)
    for c in range(NCORES):
        h = np.asarray(res.results[c]["hist"]).reshape(P, T, W)[:, :, DL:DL + C]
        flat = h.transpose(1, 0, 2).reshape(T, CORE_SLICE)
        out[:, c * OUT:(c + 1) * OUT] = (
            flat[:, HALO:HALO + OUT].astype(f32) * inv)
    return out, res


def kernel(t_steps, x_grid, initial_I, grid1, spline_w1, base_w1,
           grid2, spline_w2, base_w2, diff_param):
    out, _ = _run(dict(
        t_steps=t_steps, x_grid=x_grid, initial_I=initial_I,
        grid1=grid1, spline_w1=spline_w1, base_w1=base_w1,
        grid2=grid2, spline_w2=spline_w2, base_w2=base_w2,
        diff_param=diff_param))
    return out


# revision 6
# speedup vs baseline: 1.0893x; 1.0893x over previous
"""Trainium2 Bass kernel for nn_DiffPhysKAN.

Reaction-diffusion PDE (SIR-like) explicitly time-stepped T=100 times over a
1D grid of N=500000 points, with per-step beta(t) from a tiny KAN network and
a learned diffusion coefficient.

Strategy (v2 — u32 fixed-point state):
  - beta(t)/diff/dt/dx are tiny host-side scalar computations (T=100 values);
    they are baked into the device program as per-step immediates.
  - The spatial grid is sharded over 8 NeuronCores (1D domain decomposition).
    The replicate-boundary stencil is exactly a mirror (Neumann) boundary, so
    the host mirror-pads the initial condition; each core gets its 62500-col
    chunk plus 110-element halos and runs all 100 steps with ZERO collectives
    (ghost-zone trick: errors from stale halos advance 1 element/step and
    never reach the output region).
  - The state is kept in uint32 fixed point, J = I * (2^32-1)/10, so that the
    DVE's saturating f32->u32 write conversion performs clip(I,0,10) for
    free: J'=0 at I=0 and J'=2^32-1 at I=10 exactly. One custom 8-block DVE
    op per step computes
        S = a*(L + R) + M*(c1 - b*M);  relu;  saturating u32 round
    (a = dt*diff/dx^2 in shared I/J units, b = dt*beta_t/S32,
    c1 = 1 - 2a - dt + dt*beta_t), with the left tap L synthesized from the
    center stream by the swap-flop delay trick. u32 quantization (~5e-6 per
    step after f32 rounding) tracks the f32 reference as well as a pure-f32
    kernel does (sim: rel err 1.6e-6).
  - The ACT (scalar) engine, otherwise idle, converts each new state's 490
    data cols to u16 history (x 65535/(2^32-1)) into a persistent SBUF tile
    [128, T*W]; the Sync engine ships it to DRAM in multi-step contiguous
    chunks (~1.4 MB per dma_start, one fat descriptor per partition).
  - Partition-level ghosts are refreshed every 20 steps by two SBUF->SBUF
    DMAs shifted by one partition (staged 4 steps early), installed with two
    cheap same-engine DVE copies so the DVE never waits on a DMA.
"""

import sys

for _p in ("/opt/trn_rl_repo", "/root/.axon_site/_ro/trn_rl_repo"):
    if _p not in sys.path:
        sys.path.append(_p)

import numpy as np

f32 = np.float32
f64 = np.float64

# ---- problem/layout constants (hardcoded per contest contract) ----
T = 100
N = 500000
NCORES = 8
OUT = N // NCORES        # 62500 output cols per core
P = 128                  # SBUF partitions
C = 490                  # data cols per partition (128*490 = 62720 per core)
CORE_SLICE = P * C       # 62720
HALO = (CORE_SLICE - OUT) // 2   # 110 (>= T=100 needed)
DL = 28                  # left ghost cols
DR = 28                  # right ghost cols (W even -> 4B-aligned u16 rows)
W = DL + C + DR          # 546
PAD_L = HALO + DL        # host mirror-pad widths
PAD_R = HALO + DR
REFRESH_EVERY = 20       # ghost refresh period (staleness 4 + fronts < DL/DR)

UMAX = 4294967295.0
S32 = UMAX / 10.0                  # J = I * S32 (f64 scale on host)
C16 = float(np.float32(65535.0 / UMAX))   # u16 out = sat_round(f32(J) * C16)
S16 = 6553.5                       # I = u16 / S16

# DMA chunk sizes (steps per hist dma_start), tapered so the final chunks
# expose minimal tail latency after the last compute step. The history lives
# in SBUF for the whole run (each range written once), so chunk DMAs have no
# WAR hazards and block nothing; they ride the otherwise-idle GpSimd (SWDGE)
# queue, keeping the Sync HWDGE FIFO free for the small ghost-staging DMAs.
CHUNKS = [12, 12, 12, 12, 12, 12, 12, 8, 4, 2, 1, 1]
assert sum(CHUNKS) == T

# ---------------------------------------------------------------- host math


def _softplus(x):
    x = x.astype(f32)
    return (np.maximum(x, 0) + np.log1p(np.exp(-np.abs(x), dtype=f32), dtype=f32)).astype(f32)


def _kan_layer(x, grid, spline_w, base_w):
    x = x.astype(f32)
    base = x @ base_w.T.astype(f32)
    basis = np.exp(-((x[:, :, None] - grid[None, None, :]) ** 2) * f32(10.0), dtype=f32)
    basis = basis.reshape(x.shape[0], -1)
    return (base + basis @ spline_w).astype(f32)


def _host_params(t_steps, x_grid, grid1, spline_w1, base_w1, grid2, spline_w2,
                 base_w2, diff_param):
    h = _kan_layer(t_steps, grid1, spline_w1, base_w1)
    h = _kan_layer(h, grid2, spline_w2, base_w2)
    betas = np.clip(_softplus(h), 0.0, 20.0).astype(f32).reshape(-1)
    diff = np.clip(_softplus(diff_param), 0.0, 1.0).astype(f32)[0]
    dt = f32(t_steps[1, 0] - t_steps[0, 0])
    dx = f32(x_grid[1] - x_grid[0])
    a = f32(np.float64(dt) * np.float64(diff) / (np.float64(dx) ** 2))
    b_all = [f32(np.float64(dt) * np.float64(b)) for b in betas]
    c1_all = [f32(1.0 - 2 * np.float64(a) - np.float64(dt) + np.float64(b)) for b in b_all]
    return a, b_all, c1_all


# ------------------------------------------------------- custom DVE op

_OPS_CACHE = {}


def _get_custom_ops():
    """Register PDE_FUSED_S: a hand-written 8-block DVE micro-op computing
        S[e] = relu(a*(L + R) + M*(c1 - b*M))
    in ONE pass, where M = in0 (center view), R = in1 (right view) and the
    left tap L = M delayed by one element, synthesized with the swap flop
    (block0 BYPASS latches operand B; CURR_SWAP_OUT reads the previous
    element's value). Consts: C0=b (s0), C1=c1 (s1), C2=a (imm2).
    out[0] is garbage (uninitialized swap flop) — it lands in a ghost
    column and never reaches the output region. With a uint32 output AP the
    write conversion saturates at [0, 2^32-1], providing the upper clip."""
    if _OPS_CACHE:
        return _OPS_CACHE["S"]
    import concourse.dve_ops as D
    from concourse.dve_spec import Spec, Src0, Src1, C0, C1, C2
    from concourse.dve_uop import (UopConfig, DveOpSpec, InpSel, AluInp, AluOp,
                                   OutSel, OutPath, Trigger, DelayInp)
    ENABLE = 1

    name = "PDE_FUSED_S"
    for op in D.OPS:
        if op.name == name:
            _OPS_CACHE["S"] = op
            return op

    u = UopConfig()
    u.enable_input(InpSel.SRC_0, 1)      # M-view   -> chain0 feed
    u.enable_input(InpSel.SRC_1, 2)      # R-view   -> chain1 feed
    u.enable_input(InpSel.CONST_0, 3)    # b        -> chain2 feed
    u.enable_input(InpSel.CONST_1, 4)    # c1       -> chain3 feed
    u.enable_input(InpSel.CONST_2, 5)    # a        -> chain4 feed
    u.enable_input(InpSel.ZERO, 6)       # 0        -> chain5 feed
    u.require_inp0 = ENABLE
    u.require_inp1 = ENABLE
    u.trigger = (Trigger.SRC_TENSOR_DONE, Trigger.NONE, Trigger.NONE)
    dp = u.datapath_config
    # b0: L = delayed M  (BYPASS passes A=CURR_SWAP_OUT; swap latches B=M)
    dp[0].enable_alu(AluOp.BYPASS, AluInp.CURR_SWAP_OUT, AluInp.PREV_DELAY_0)
    dp[0].swap_enable = ENABLE
    dp[0].pass_through_delay(0, 1, 2, 3, 4, 5)
    # b1: u = L + R
    dp[1].enable_alu(AluOp.ADD, AluInp.PREV_ALU_OUT, AluInp.PREV_DELAY_1)
    dp[1].pass_through_delay(0, 2, 3, 4, 5)
    # b2: t1 = M * b ; park u in chain1
    dp[2].enable_alu(AluOp.MULTIPLY, AluInp.PREV_DELAY_0, AluInp.PREV_DELAY_2)
    dp[2].enable_delay_from_src(DelayInp.PREV_ALU_OUT, 1)
    dp[2].pass_through_delay(0, 3, 4, 5)
    # b3: t2 = c1 - t1
    dp[3].enable_alu(AluOp.SUBTRACT, AluInp.PREV_DELAY_3, AluInp.PREV_ALU_OUT)
    dp[3].pass_through_delay(0, 1, 4, 5)
    # b4: Q = t2 * M
    dp[4].enable_alu(AluOp.MULTIPLY, AluInp.PREV_ALU_OUT, AluInp.PREV_DELAY_0)
    dp[4].pass_through_delay(1, 4, 5)
    # b5: au = u * a ; park Q in chain0
    dp[5].enable_alu(AluOp.MULTIPLY, AluInp.PREV_DELAY_1, AluInp.PREV_DELAY_4)
    dp[5].enable_delay_from_src(DelayInp.PREV_ALU_OUT, 0)
    dp[5].pass_through_delay(5)
    # b6: S = au + Q
    dp[6].enable_alu(AluOp.ADD, AluInp.PREV_ALU_OUT, AluInp.PREV_DELAY_0)
    dp[6].pass_through_delay(5)
    # b7: max(S, 0) — lower clip (redundant with u32 saturation, kept)
    dp[7].enable_alu(AluOp.MAX, AluInp.PREV_ALU_OUT, AluInp.PREV_DELAY_5)
    u.enable_output(OutSel.ALU_OUT, OutPath.WR0_LO)

    def _ref(in0, in1, s0, s1, imm2):
        in0 = in0.astype(np.float32)
        in1 = in1.astype(np.float32)
        L = np.concatenate([in0[:, :1], in0[:, :-1]], axis=1)
        return np.maximum(
            imm2 * (L + in1) + in0 * (s1 - in0 * s0), 0.0).astype(np.float32)

    spec = Spec(body=(Src0 + Src1) * C2 + Src0 * (C1 - Src0 * C0),
                reference=_ref)
    op = D.DveOp(name, spec, subdim=False, uops_sha={})
    D.OPS.append(op)
    D._SUB_OPCODE_FOR_NAME[name] = D._CUSTOM_DVE_ROW_BASE + len(D.OPS) - 1
    D.CUSTOM_DVE_SPECS[name] = spec
    opspec = DveOpSpec(name=name, opcode=D._SUB_OPCODE_FOR_NAME[name],
                       uops=[u], rd1_en=True)
    for ver in ("v3", "v4"):
        D._COMPILE_CACHE[(name, ver)] = opspec
    _OPS_CACHE["S"] = op
    return op


# ------------------------------------------------------- device program


def _build_program(a, b_all, c1_all):
    from concourse import bacc, mybir
    from concourse.tile import TileContext

    op_s = _get_custom_ops()
    nc = bacc.Bacc(None, target_bir_lowering=False)
    x0 = nc.declare_dram_parameter("x0", [P, W], mybir.dt.uint32, isOutput=False)
    hist = nc.declare_dram_parameter("hist", [P, T * W], mybir.dt.uint16,
                                     isOutput=True)

    # per-step J-unit constants: b_t scaled into u32 units, c1 unchanged
    b32_all = [float(np.float32(np.float64(b) / S32)) for b in b_all]
    c1f_all = [float(c) for c in c1_all]
    af = float(a)

    with TileContext(nc) as tc:
        with tc.tile_pool(name="x", bufs=8) as xpool, \
             tc.tile_pool(name="h", bufs=1) as hpool, \
             tc.tile_pool(name="g", bufs=2) as gpool:
            H = hpool.tile([P, T * W], mybir.dt.uint16)
            X = xpool.tile([P, W], mybir.dt.uint32)
            nc.sync.dma_start(out=X[:, :], in_=x0[:, :])
            pending = None
            done = 0
            nxt = 0
            for t in range(T):
                Xn = xpool.tile([P, W], mybir.dt.uint32)
                nc.vector._custom_dve(op_s, out=Xn[:, 2:W - 1],
                                      in0=X[:, 2:W - 1], in1=X[:, 3:W],
                                      s0=b32_all[t], s1=c1f_all[t],
                                      imm2=af)
                # ACT: u32 state -> u16 history (saturating round on write)
                nc.scalar.mul(H[:, t * W + DL:t * W + DL + C],
                              Xn[:, DL:DL + C], C16)
                X = Xn
                # Ghost refresh: stage partition-shifted halo data via the Sync
                # HWDGE queue four steps early (even staleness matches the
                # saturated field's period-2 oscillation; garbage fronts stay
                # below DL/DR), then install with two cheap same-engine DVE
                # copies so the DVE never waits on a DMA.
                if (t + 5) % REFRESH_EVERY == 0 and (t + 5) < T:
                    gl = gpool.tile([P, DL], mybir.dt.uint32, tag="gl")
                    gr = gpool.tile([P, DR], mybir.dt.uint32, tag="gr")
                    nc.sync.dma_start(out=gl[1:P, :], in_=X[0:P - 1, C:C + DL])
                    nc.sync.dma_start(out=gr[0:P - 1, :], in_=X[1:P, DL:DL + DR])
                    pending = (gl, gr)
                if (t + 1) % REFRESH_EVERY == 0 and (t + 1) < T:
                    gl, gr = pending
                    nc.vector.tensor_copy(X[:, 0:DL], gl[:, :])
                    nc.vector.tensor_copy(X[:, C + DL:W], gr[:, :])
                # chunked history DMA (contiguous per partition) on the idle
                # GpSimd SWDGE queue; nothing ever waits on these except the
                # kernel end.
                if done + CHUNKS[nxt] == t + 1:
                    lo, hi = done * W, (t + 1) * W
                    nc.gpsimd.dma_start(out=hist[:, lo:hi], in_=H[:, lo:hi])
                    done = t + 1
                    nxt += 1
    nc.finalize()
    return nc


# ------------------------------------------------------------- entry points


def _run(inputs, trace=False, trace_kwargs=None):
    from concourse.bass_utils import run_bass_kernel_spmd

    t_steps = np.asarray(inputs["t_steps"], f32)
    x_grid = np.asarray(inputs["x_grid"], f32)
    initial_I = np.asarray(inputs["initial_I"], f32)
    a, b_all, c1_all = _host_params(
        t_steps, x_grid,
        np.asarray(inputs["grid1"], f32), np.asarray(inputs["spline_w1"], f32),
        np.asarray(inputs["base_w1"], f32),
        np.asarray(inputs["grid2"], f32), np.asarray(inputs["spline_w2"], f32),
        np.asarray(inputs["base_w2"], f32), np.asarray(inputs["diff_param"], f32))

    G = np.pad(initial_I, (PAD_L, PAD_R), mode="symmetric")
    J = np.rint(G.astype(f64) * S32).astype(np.uint32)
    sw = np.lib.stride_tricks.sliding_window_view(J, W)
    row0 = np.arange(P) * C
    in_maps = []
    for c in range(NCORES):
        tile = np.ascontiguousarray(sw[c * OUT + row0], dtype=np.uint32)
        in_maps.append({"x0": tile})

    nc = _build_program(a, b_all, c1_all)
    res = run_bass_kernel_spmd(nc, in_maps, core_ids=list(range(NCORES)),
                               trace=trace, trace_kwargs=trace_kwargs or {})

    out = np.empty((T, N), f32)
    inv = f32(1.0) / f32(S16)
    for c in range(NCORES):
        h = np.asarray(res.results[c]["hist"]).reshape(P, T, W)[:, :, DL:DL + C]
        flat = h.transpose(1, 0, 2).reshape(T, CORE_SLICE)
        out[:, c * OUT:(c + 1) * OUT] = (
            flat[:, HALO:HALO + OUT].astype(f32) * inv)
    return out, res


def kernel(t_steps, x_grid, initial_I, grid1, spline_w1, base_w1,
           grid2, spline_w2, base_w2, diff_param):
    out, _ = _run(dict(
        t_steps=t_steps, x_grid=x_grid, initial_I=initial_I,
        grid1=grid1, spline_w1=spline_w1, base_w1=base_w1,
        grid2=grid2, spline_w2=spline_w2, base_w2=base_w2,
        diff_param=diff_param))
    return out


# revision 8
# speedup vs baseline: 1.1311x; 1.0384x over previous
"""Trainium2 Bass kernel for nn_DiffPhysKAN.

Reaction-diffusion PDE (SIR-like) explicitly time-stepped T=100 times over a
1D grid of N=500000 points, with per-step beta(t) from a tiny KAN network and
a learned diffusion coefficient.

Strategy (v2 — u32 fixed-point state):
  - beta(t)/diff/dt/dx are tiny host-side scalar computations (T=100 values);
    they are baked into the device program as per-step immediates.
  - The spatial grid is sharded over 8 NeuronCores (1D domain decomposition).
    The replicate-boundary stencil is exactly a mirror (Neumann) boundary, so
    the host mirror-pads the initial condition; each core gets its 62500-col
    chunk plus 110-element halos and runs all 100 steps with ZERO collectives
    (ghost-zone trick: errors from stale halos advance 1 element/step and
    never reach the output region).
  - The state is kept in uint32 fixed point, J = I * (2^32-1)/10, so that the
    DVE's saturating f32->u32 write conversion performs clip(I,0,10) for
    free: J'=0 at I=0 and J'=2^32-1 at I=10 exactly. One custom 8-block DVE
    op per step computes
        S = a*(L + R) + M*(c1 - b*M);  relu;  saturating u32 round
    (a = dt*diff/dx^2 in shared I/J units, b = dt*beta_t/S32,
    c1 = 1 - 2a - dt + dt*beta_t), with the left tap L synthesized from the
    center stream by the swap-flop delay trick. u32 quantization (~5e-6 per
    step after f32 rounding) tracks the f32 reference as well as a pure-f32
    kernel does (sim: rel err 1.6e-6).
  - The ACT (scalar) engine, otherwise idle, converts each new state's 490
    data cols to u16 history (x 65535/(2^32-1)) into a persistent SBUF tile
    [128, T*W]; the Sync engine ships it to DRAM in multi-step contiguous
    chunks (~1.4 MB per dma_start, one fat descriptor per partition).
  - Partition-level ghosts are refreshed every 20 steps by two SBUF->SBUF
    DMAs shifted by one partition (staged 4 steps early), installed with two
    cheap same-engine DVE copies so the DVE never waits on a DMA.
"""

import sys

for _p in ("/opt/trn_rl_repo", "/root/.axon_site/_ro/trn_rl_repo"):
    if _p not in sys.path:
        sys.path.append(_p)

import numpy as np

f32 = np.float32
f64 = np.float64

# ---- problem/layout constants (hardcoded per contest contract) ----
T = 100
N = 500000
NCORES = 8
OUT = N // NCORES        # 62500 output cols per core
P = 128                  # SBUF partitions
C = 490                  # data cols per partition (128*490 = 62720 per core)
CORE_SLICE = P * C       # 62720
HALO = (CORE_SLICE - OUT) // 2   # 110 (>= T=100 needed)
DL = 28                  # left ghost cols
DR = 28                  # right ghost cols (W even -> 4B-aligned u16 rows)
W = DL + C + DR          # 546
PAD_L = HALO + DL        # host mirror-pad widths
PAD_R = HALO + DR
REFRESH_EVERY = 20       # ghost refresh period (staleness 4 + fronts < DL/DR)

UMAX = 4294967295.0
S32 = UMAX / 10.0                  # J = I * S32 (f64 scale on host)
C16 = float(np.float32(65535.0 / UMAX))   # u16 out = sat_round(f32(J) * C16)
S16 = 6553.5                       # I = u16 / S16

# DMA chunk sizes (steps per hist dma_start), tapered so the final chunks
# expose minimal tail latency after the last compute step. The history lives
# in SBUF for the whole run (each range written once), so chunk DMAs have no
# WAR hazards and block nothing; they ride the otherwise-idle GpSimd (SWDGE)
# queue, keeping the Sync HWDGE FIFO free for the small ghost-staging DMAs.
CHUNKS = [12, 12, 12, 12, 12, 12, 12, 8, 4, 2, 1, 1]
assert sum(CHUNKS) == T

# ---------------------------------------------------------------- host math


def _softplus(x):
    x = x.astype(f32)
    return (np.maximum(x, 0) + np.log1p(np.exp(-np.abs(x), dtype=f32), dtype=f32)).astype(f32)


def _kan_layer(x, grid, spline_w, base_w):
    x = x.astype(f32)
    base = x @ base_w.T.astype(f32)
    basis = np.exp(-((x[:, :, None] - grid[None, None, :]) ** 2) * f32(10.0), dtype=f32)
    basis = basis.reshape(x.shape[0], -1)
    return (base + basis @ spline_w).astype(f32)


def _host_params(t_steps, x_grid, grid1, spline_w1, base_w1, grid2, spline_w2,
                 base_w2, diff_param):
    h = _kan_layer(t_steps, grid1, spline_w1, base_w1)
    h = _kan_layer(h, grid2, spline_w2, base_w2)
    betas = np.clip(_softplus(h), 0.0, 20.0).astype(f32).reshape(-1)
    diff = np.clip(_softplus(diff_param), 0.0, 1.0).astype(f32)[0]
    dt = f32(t_steps[1, 0] - t_steps[0, 0])
    dx = f32(x_grid[1] - x_grid[0])
    a = f32(np.float64(dt) * np.float64(diff) / (np.float64(dx) ** 2))
    b_all = [f32(np.float64(dt) * np.float64(b)) for b in betas]
    c1_all = [f32(1.0 - 2 * np.float64(a) - np.float64(dt) + np.float64(b)) for b in b_all]
    return a, b_all, c1_all


# ------------------------------------------------------- custom DVE op

_OPS_CACHE = {}


def _get_custom_ops():
    """Register PDE_FUSED_S: a hand-written 8-block DVE micro-op computing
        S[e] = relu(a*(L + R) + M*(c1 - b*M))
    in ONE pass, where M = in0 (center view), R = in1 (right view) and the
    left tap L = M delayed by one element, synthesized with the swap flop
    (block0 BYPASS latches operand B; CURR_SWAP_OUT reads the previous
    element's value). Consts: C0=b (s0), C1=c1 (s1), C2=a (imm2).
    out[0] is garbage (uninitialized swap flop) — it lands in a ghost
    column and never reaches the output region. With a uint32 output AP the
    write conversion saturates at [0, 2^32-1], providing the upper clip."""
    if _OPS_CACHE:
        return _OPS_CACHE["S"]
    import concourse.dve_ops as D
    from concourse.dve_spec import Spec, Src0, Src1, C0, C1, C2
    from concourse.dve_uop import (UopConfig, DveOpSpec, InpSel, AluInp, AluOp,
                                   OutSel, OutPath, Trigger, DelayInp)
    ENABLE = 1

    name = "PDE_FUSED_S"
    for op in D.OPS:
        if op.name == name:
            _OPS_CACHE["S"] = op
            return op

    u = UopConfig()
    u.enable_input(InpSel.SRC_0, 1)      # M-view   -> chain0 feed
    u.enable_input(InpSel.SRC_1, 2)      # R-view   -> chain1 feed
    u.enable_input(InpSel.CONST_0, 3)    # b        -> chain2 feed
    u.enable_input(InpSel.CONST_1, 4)    # c1       -> chain3 feed
    u.enable_input(InpSel.CONST_2, 5)    # a        -> chain4 feed
    u.enable_input(InpSel.ZERO, 6)       # 0        -> chain5 feed
    u.require_inp0 = ENABLE
    u.require_inp1 = ENABLE
    u.trigger = (Trigger.SRC_TENSOR_DONE, Trigger.NONE, Trigger.NONE)
    dp = u.datapath_config
    # b0: L = delayed M  (BYPASS passes A=CURR_SWAP_OUT; swap latches B=M)
    dp[0].enable_alu(AluOp.BYPASS, AluInp.CURR_SWAP_OUT, AluInp.PREV_DELAY_0)
    dp[0].swap_enable = ENABLE
    dp[0].pass_through_delay(0, 1, 2, 3, 4, 5)
    # b1: u = L + R
    dp[1].enable_alu(AluOp.ADD, AluInp.PREV_ALU_OUT, AluInp.PREV_DELAY_1)
    dp[1].pass_through_delay(0, 2, 3, 4, 5)
    # b2: t1 = M * b ; park u in chain1
    dp[2].enable_alu(AluOp.MULTIPLY, AluInp.PREV_DELAY_0, AluInp.PREV_DELAY_2)
    dp[2].enable_delay_from_src(DelayInp.PREV_ALU_OUT, 1)
    dp[2].pass_through_delay(0, 3, 4, 5)
    # b3: t2 = c1 - t1
    dp[3].enable_alu(AluOp.SUBTRACT, AluInp.PREV_DELAY_3, AluInp.PREV_ALU_OUT)
    dp[3].pass_through_delay(0, 1, 4, 5)
    # b4: Q = t2 * M
    dp[4].enable_alu(AluOp.MULTIPLY, AluInp.PREV_ALU_OUT, AluInp.PREV_DELAY_0)
    dp[4].pass_through_delay(1, 4, 5)
    # b5: au = u * a ; park Q in chain0
    dp[5].enable_alu(AluOp.MULTIPLY, AluInp.PREV_DELAY_1, AluInp.PREV_DELAY_4)
    dp[5].enable_delay_from_src(DelayInp.PREV_ALU_OUT, 0)
    dp[5].pass_through_delay(5)
    # b6: S = au + Q
    dp[6].enable_alu(AluOp.ADD, AluInp.PREV_ALU_OUT, AluInp.PREV_DELAY_0)
    dp[6].pass_through_delay(5)
    # b7: max(S, 0) — lower clip (redundant with u32 saturation, kept)
    dp[7].enable_alu(AluOp.MAX, AluInp.PREV_ALU_OUT, AluInp.PREV_DELAY_5)
    u.enable_output(OutSel.ALU_OUT, OutPath.WR0_LO)

    def _ref(in0, in1, s0, s1, imm2):
        in0 = in0.astype(np.float32)
        in1 = in1.astype(np.float32)
        L = np.concatenate([in0[:, :1], in0[:, :-1]], axis=1)
        return np.maximum(
            imm2 * (L + in1) + in0 * (s1 - in0 * s0), 0.0).astype(np.float32)

    spec = Spec(body=(Src0 + Src1) * C2 + Src0 * (C1 - Src0 * C0),
                reference=_ref)
    op = D.DveOp(name, spec, subdim=False, uops_sha={})
    D.OPS.append(op)
    D._SUB_OPCODE_FOR_NAME[name] = D._CUSTOM_DVE_ROW_BASE + len(D.OPS) - 1
    D.CUSTOM_DVE_SPECS[name] = spec
    opspec = DveOpSpec(name=name, opcode=D._SUB_OPCODE_FOR_NAME[name],
                       uops=[u], rd1_en=True)
    for ver in ("v3", "v4"):
        D._COMPILE_CACHE[(name, ver)] = opspec
    _OPS_CACHE["S"] = op
    return op


# ------------------------------------------------------- device program


def _build_program(a, b_all, c1_all):
    from concourse import bacc, mybir
    from concourse.tile import TileContext

    op_s = _get_custom_ops()
    nc = bacc.Bacc(None, target_bir_lowering=False)
    x0 = nc.declare_dram_parameter("x0", [P, W], mybir.dt.uint32, isOutput=False)
    hist = nc.declare_dram_parameter("hist", [P, T * W], mybir.dt.uint16,
                                     isOutput=True)

    # per-step J-unit constants: b_t scaled into u32 units, c1 unchanged
    b32_all = [float(np.float32(np.float64(b) / S32)) for b in b_all]
    c1f_all = [float(c) for c in c1_all]
    af = float(a)

    with TileContext(nc) as tc:
        with tc.tile_pool(name="x", bufs=10) as xpool, \
             tc.tile_pool(name="h", bufs=1) as hpool, \
             tc.tile_pool(name="g", bufs=2) as gpool:
            H = hpool.tile([P, T * W], mybir.dt.uint16)
            X = xpool.tile([P, W], mybir.dt.uint32)
            nc.sync.dma_start(out=X[:, :], in_=x0[:, :])
            pending = None
            done = 0
            nxt = 0
            for t in range(T):
                Xn = xpool.tile([P, W], mybir.dt.uint32)
                nc.vector._custom_dve(op_s, out=Xn[:, 2:W - 1],
                                      in0=X[:, 2:W - 1], in1=X[:, 3:W],
                                      s0=b32_all[t], s1=c1f_all[t],
                                      imm2=af)
                # ACT: u32 state -> u16 history (saturating round on write)
                nc.scalar.mul(H[:, t * W + DL:t * W + DL + C],
                              Xn[:, DL:DL + C], C16)
                X = Xn
                # Ghost refresh: stage partition-shifted halo data via the Sync
                # HWDGE queue six steps early (even staleness matches the
                # saturated field's period-2 oscillation; garbage fronts stay
                # below DL/DR), then install with two cheap same-engine DVE
                # copies so the DVE never waits on a DMA.
                if (t + 7) % REFRESH_EVERY == 0 and (t + 7) < T:
                    gl = gpool.tile([P, DL], mybir.dt.uint32, tag="gl")
                    gr = gpool.tile([P, DR], mybir.dt.uint32, tag="gr")
                    nc.sync.dma_start(out=gl[1:P, :], in_=X[0:P - 1, C:C + DL])
                    nc.sync.dma_start(out=gr[0:P - 1, :], in_=X[1:P, DL:DL + DR])
                    pending = (gl, gr)
                if (t + 1) % REFRESH_EVERY == 0 and (t + 1) < T:
                    gl, gr = pending
                    nc.vector.tensor_copy(X[:, 0:DL], gl[:, :])
                    nc.vector.tensor_copy(X[:, C + DL:W], gr[:, :])
                # chunked history DMA (contiguous per partition) on the idle
                # GpSimd SWDGE queue; nothing ever waits on these except the
                # kernel end.
                if done + CHUNKS[nxt] == t + 1:
                    lo, hi = done * W, (t + 1) * W
                    nc.gpsimd.dma_start(out=hist[:, lo:hi], in_=H[:, lo:hi])
                    done = t + 1
                    nxt += 1
    nc.finalize()
    return nc


# ------------------------------------------------------------- entry points


def _run(inputs, trace=False, trace_kwargs=None):
    from concourse.bass_utils import run_bass_kernel_spmd

    t_steps = np.asarray(inputs["t_steps"], f32)
    x_grid = np.asarray(inputs["x_grid"], f32)
    initial_I = np.asarray(inputs["initial_I"], f32)
    a, b_all, c1_all = _host_params(
        t_steps, x_grid,
        np.asarray(inputs["grid1"], f32), np.asarray(inputs["spline_w1"], f32),
        np.asarray(inputs["base_w1"], f32),
        np.asarray(inputs["grid2"], f32), np.asarray(inputs["spline_w2"], f32),
        np.asarray(inputs["base_w2"], f32), np.asarray(inputs["diff_param"], f32))

    G = np.pad(initial_I, (PAD_L, PAD_R), mode="symmetric")
    J = np.rint(G.astype(f64) * S32).astype(np.uint32)
    sw = np.lib.stride_tricks.sliding_window_view(J, W)
    row0 = np.arange(P) * C
    in_maps = []
    for c in range(NCORES):
        tile = np.ascontiguousarray(sw[c * OUT + row0], dtype=np.uint32)
        in_maps.append({"x0": tile})

    nc = _build_program(a, b_all, c1_all)
    res = run_bass_kernel_spmd(nc, in_maps, core_ids=list(range(NCORES)),
                               trace=trace, trace_kwargs=trace_kwargs or {})

    out = np.empty((T, N), f32)
    inv = f32(1.0) / f32(S16)
    for c in range(NCORES):
        h = np.asarray(res.results[c]["hist"]).reshape(P, T, W)[:, :, DL:DL + C]
        flat = h.transpose(1, 0, 2).reshape(T, CORE_SLICE)
        out[:, c * OUT:(c + 1) * OUT] = (
            flat[:, HALO:HALO + OUT].astype(f32) * inv)
    return out, res


def kernel(t_steps, x_grid, initial_I, grid1, spline_w1, base_w1,
           grid2, spline_w2, base_w2, diff_param):
    out, _ = _run(dict(
        t_steps=t_steps, x_grid=x_grid, initial_I=initial_I,
        grid1=grid1, spline_w1=spline_w1, base_w1=base_w1,
        grid2=grid2, spline_w2=spline_w2, base_w2=base_w2,
        diff_param=diff_param))
    return out


# revision 12
# speedup vs baseline: 1.2801x; 1.1317x over previous
"""Trainium2 Bass kernel for nn_DiffPhysKAN.

Reaction-diffusion PDE (SIR-like) explicitly time-stepped T=100 times over a
1D grid of N=500000 points, with per-step beta(t) from a tiny KAN network and
a learned diffusion coefficient.

Strategy (v2 — u32 fixed-point state):
  - beta(t)/diff/dt/dx are tiny host-side scalar computations (T=100 values);
    they are baked into the device program as per-step immediates.
  - The spatial grid is sharded over 8 NeuronCores (1D domain decomposition).
    The replicate-boundary stencil is exactly a mirror (Neumann) boundary, so
    the host mirror-pads the initial condition; each core gets its 62500-col
    chunk plus 110-element halos and runs all 100 steps with ZERO collectives
    (ghost-zone trick: errors from stale halos advance 1 element/step and
    never reach the output region).
  - The state is kept in uint32 fixed point, J = I * (2^32-1)/10, so that the
    DVE's saturating f32->u32 write conversion performs clip(I,0,10) for
    free: J'=0 at I=0 and J'=2^32-1 at I=10 exactly. One custom 8-block DVE
    op per step computes
        S = a*(L + R) + M*(c1 - b*M);  relu;  saturating u32 round
    (a = dt*diff/dx^2 in shared I/J units, b = dt*beta_t/S32,
    c1 = 1 - 2a - dt + dt*beta_t), with the left tap L synthesized from the
    center stream by the swap-flop delay trick. u32 quantization (~5e-6 per
    step after f32 rounding) tracks the f32 reference as well as a pure-f32
    kernel does (sim: rel err 1.6e-6).
  - The ACT (scalar) engine, otherwise idle, converts each new state's 490
    data cols to u16 history (x 65535/(2^32-1)) into a persistent SBUF tile
    [128, T*W]; the Sync engine ships it to DRAM in multi-step contiguous
    chunks (~1.4 MB per dma_start, one fat descriptor per partition).
  - Partition-level ghosts are refreshed every 20 steps by two SBUF->SBUF
    DMAs shifted by one partition (staged 4 steps early), installed with two
    cheap same-engine DVE copies so the DVE never waits on a DMA.
"""

import sys

for _p in ("/opt/trn_rl_repo", "/root/.axon_site/_ro/trn_rl_repo"):
    if _p not in sys.path:
        sys.path.append(_p)

import numpy as np

f32 = np.float32
f64 = np.float64

# ---- problem/layout constants (hardcoded per contest contract) ----
T = 100
N = 500000
NCORES = 8
OUT = N // NCORES        # 62500 output cols per core
P = 128                  # SBUF partitions
C = 490                  # data cols per partition (128*490 = 62720 per core)
CORE_SLICE = P * C       # 62720
HALO = (CORE_SLICE - OUT) // 2   # 110 (>= T=100 needed)
DL = 28                  # left ghost cols
DR = 28                  # right ghost cols (W even -> 4B-aligned u16 rows)
W = DL + C + DR          # 546
PAD_L = HALO + DL        # host mirror-pad widths
PAD_R = HALO + DR
REFRESH_EVERY = 24       # ghost refresh period (staleness 6 + fronts < DL/DR;
                         # multiple of the 12-step chunk period so staging DMAs
                         # never share the SDMA engines with a history chunk)

UMAX = 4294967295.0
S32 = UMAX / 10.0                  # J = I * S32 (f64 scale on host)
C16 = float(np.float32(65535.0 / UMAX))   # u16 out = sat_round(f32(J) * C16)
S16 = 6553.5                       # I = u16 / S16

# DMA chunk sizes (steps per hist dma_start), tapered so the final chunks
# expose minimal tail latency after the last compute step. The history lives
# in SBUF for the whole run (each range written once), so chunk DMAs have no
# WAR hazards and block nothing; they ride the otherwise-idle GpSimd (SWDGE)
# queue, keeping the Sync HWDGE FIFO free for the small ghost-staging DMAs.
CHUNKS = [12, 12, 12, 12, 12, 12, 12, 10, 3, 2, 1]
assert sum(CHUNKS) == T

# ---------------------------------------------------------------- host math


def _softplus(x):
    x = x.astype(f32)
    return (np.maximum(x, 0) + np.log1p(np.exp(-np.abs(x), dtype=f32), dtype=f32)).astype(f32)


def _kan_layer(x, grid, spline_w, base_w):
    x = x.astype(f32)
    base = x @ base_w.T.astype(f32)
    basis = np.exp(-((x[:, :, None] - grid[None, None, :]) ** 2) * f32(10.0), dtype=f32)
    basis = basis.reshape(x.shape[0], -1)
    return (base + basis @ spline_w).astype(f32)


def _host_params(t_steps, x_grid, grid1, spline_w1, base_w1, grid2, spline_w2,
                 base_w2, diff_param):
    h = _kan_layer(t_steps, grid1, spline_w1, base_w1)
    h = _kan_layer(h, grid2, spline_w2, base_w2)
    betas = np.clip(_softplus(h), 0.0, 20.0).astype(f32).reshape(-1)
    diff = np.clip(_softplus(diff_param), 0.0, 1.0).astype(f32)[0]
    dt = f32(t_steps[1, 0] - t_steps[0, 0])
    dx = f32(x_grid[1] - x_grid[0])
    a = f32(np.float64(dt) * np.float64(diff) / (np.float64(dx) ** 2))
    b_all = [f32(np.float64(dt) * np.float64(b)) for b in betas]
    c1_all = [f32(1.0 - 2 * np.float64(a) - np.float64(dt) + np.float64(b)) for b in b_all]
    return a, b_all, c1_all


# ------------------------------------------------------- custom DVE op

_OPS_CACHE = {}


def _get_custom_ops():
    """Register PDE_FUSED_S: a hand-written 8-block DVE micro-op computing
        S[e] = relu(a*(L + R) + M*(c1 - b*M))
    in ONE pass, where M = in0 (center view), R = in1 (right view) and the
    left tap L = M delayed by one element, synthesized with the swap flop
    (block0 BYPASS latches operand B; CURR_SWAP_OUT reads the previous
    element's value). Consts: C0=b (s0), C1=c1 (s1), C2=a (imm2).
    out[0] is garbage (uninitialized swap flop) — it lands in a ghost
    column and never reaches the output region. With a uint32 output AP the
    write conversion saturates at [0, 2^32-1], providing the upper clip."""
    if _OPS_CACHE:
        return _OPS_CACHE["S"]
    import concourse.dve_ops as D
    from concourse.dve_spec import Spec, Src0, Src1, C0, C1, C2
    from concourse.dve_uop import (UopConfig, DveOpSpec, InpSel, AluInp, AluOp,
                                   OutSel, OutPath, Trigger, DelayInp)
    ENABLE = 1

    name = "PDE_FUSED_S"
    for op in D.OPS:
        if op.name == name:
            _OPS_CACHE["S"] = op
            return op

    u = UopConfig()
    u.enable_input(InpSel.SRC_0, 1)      # M-view   -> chain0 feed
    u.enable_input(InpSel.SRC_1, 2)      # R-view   -> chain1 feed
    u.enable_input(InpSel.CONST_0, 3)    # b        -> chain2 feed
    u.enable_input(InpSel.CONST_1, 4)    # c1       -> chain3 feed
    u.enable_input(InpSel.CONST_2, 5)    # a        -> chain4 feed
    u.enable_input(InpSel.ZERO, 6)       # 0        -> chain5 feed
    u.require_inp0 = ENABLE
    u.require_inp1 = ENABLE
    u.trigger = (Trigger.SRC_TENSOR_DONE, Trigger.NONE, Trigger.NONE)
    dp = u.datapath_config
    # b0: L = delayed M  (BYPASS passes A=CURR_SWAP_OUT; swap latches B=M)
    dp[0].enable_alu(AluOp.BYPASS, AluInp.CURR_SWAP_OUT, AluInp.PREV_DELAY_0)
    dp[0].swap_enable = ENABLE
    dp[0].pass_through_delay(0, 1, 2, 3, 4, 5)
    # b1: u = L + R
    dp[1].enable_alu(AluOp.ADD, AluInp.PREV_ALU_OUT, AluInp.PREV_DELAY_1)
    dp[1].pass_through_delay(0, 2, 3, 4, 5)
    # b2: t1 = M * b ; park u in chain1
    dp[2].enable_alu(AluOp.MULTIPLY, AluInp.PREV_DELAY_0, AluInp.PREV_DELAY_2)
    dp[2].enable_delay_from_src(DelayInp.PREV_ALU_OUT, 1)
    dp[2].pass_through_delay(0, 3, 4, 5)
    # b3: t2 = c1 - t1
    dp[3].enable_alu(AluOp.SUBTRACT, AluInp.PREV_DELAY_3, AluInp.PREV_ALU_OUT)
    dp[3].pass_through_delay(0, 1, 4, 5)
    # b4: Q = t2 * M
    dp[4].enable_alu(AluOp.MULTIPLY, AluInp.PREV_ALU_OUT, AluInp.PREV_DELAY_0)
    dp[4].pass_through_delay(1, 4, 5)
    # b5: au = u * a ; park Q in chain0
    dp[5].enable_alu(AluOp.MULTIPLY, AluInp.PREV_DELAY_1, AluInp.PREV_DELAY_4)
    dp[5].enable_delay_from_src(DelayInp.PREV_ALU_OUT, 0)
    dp[5].pass_through_delay(5)
    # b6: S = au + Q
    dp[6].enable_alu(AluOp.ADD, AluInp.PREV_ALU_OUT, AluInp.PREV_DELAY_0)
    dp[6].pass_through_delay(5)
    # b7: max(S, 0) — lower clip (redundant with u32 saturation, kept)
    dp[7].enable_alu(AluOp.MAX, AluInp.PREV_ALU_OUT, AluInp.PREV_DELAY_5)
    u.enable_output(OutSel.ALU_OUT, OutPath.WR0_LO)

    def _ref(in0, in1, s0, s1, imm2):
        in0 = in0.astype(np.float32)
        in1 = in1.astype(np.float32)
        L = np.concatenate([in0[:, :1], in0[:, :-1]], axis=1)
        return np.maximum(
            imm2 * (L + in1) + in0 * (s1 - in0 * s0), 0.0).astype(np.float32)

    spec = Spec(body=(Src0 + Src1) * C2 + Src0 * (C1 - Src0 * C0),
                reference=_ref)
    op = D.DveOp(name, spec, subdim=False, uops_sha={})
    D.OPS.append(op)
    D._SUB_OPCODE_FOR_NAME[name] = D._CUSTOM_DVE_ROW_BASE + len(D.OPS) - 1
    D.CUSTOM_DVE_SPECS[name] = spec
    opspec = DveOpSpec(name=name, opcode=D._SUB_OPCODE_FOR_NAME[name],
                       uops=[u], rd1_en=True)
    for ver in ("v3", "v4"):
        D._COMPILE_CACHE[(name, ver)] = opspec
    _OPS_CACHE["S"] = op
    return op


# ------------------------------------------------------- device program


def _build_program(a, b_all, c1_all):
    from concourse import bacc, mybir
    from concourse.tile import TileContext

    op_s = _get_custom_ops()
    nc = bacc.Bacc(None, target_bir_lowering=False)
    x0 = nc.declare_dram_parameter("x0", [P, W], mybir.dt.uint32, isOutput=False)
    hist = nc.declare_dram_parameter("hist", [P, T * W], mybir.dt.uint16,
                                     isOutput=True)

    # per-step J-unit constants: b_t scaled into u32 units, c1 unchanged
    b32_all = [float(np.float32(np.float64(b) / S32)) for b in b_all]
    c1f_all = [float(c) for c in c1_all]
    af = float(a)

    GRP = 4                  # states per grouped tile (1 pool-WAR wait per GRP)
    with TileContext(nc) as tc:
        with tc.tile_pool(name="x0p", bufs=1) as x0pool, \
             tc.tile_pool(name="x", bufs=3) as xpool, \
             tc.tile_pool(name="h", bufs=1) as hpool, \
             tc.tile_pool(name="g", bufs=2) as gpool:
            H = hpool.tile([P, T * W], mybir.dt.uint16)
            X0 = x0pool.tile([P, W], mybir.dt.uint32)
            nc.sync.dma_start(out=X0[:, :], in_=x0[:, :])
            X, xo = X0, 0        # current state: tile + column offset
            pending = None
            done = 0
            nxt = 0
            G = None
            for t in range(T):
                if t % GRP == 0:
                    G = xpool.tile([P, GRP * W], mybir.dt.uint32)
                no = (t % GRP) * W
                nc.vector._custom_dve(op_s, out=G[:, no + 2:no + W - 1],
                                      in0=X[:, xo + 2:xo + W - 1],
                                      in1=X[:, xo + 3:xo + W],
                                      s0=b32_all[t], s1=c1f_all[t],
                                      imm2=af)
                # ACT: u32 state -> u16 history (saturating round on write)
                nc.scalar.mul(H[:, t * W + DL:t * W + DL + C],
                              G[:, no + DL:no + DL + C], C16)
                X, xo = G, no
                # Ghost refresh: stage partition-shifted halo data via the Sync
                # HWDGE queue six steps early (even staleness matches the
                # saturated field's period-2 oscillation; garbage fronts stay
                # below DL/DR), then install with two cheap same-engine DVE
                # copies so the DVE never waits on a DMA.
                if (t + 7) % REFRESH_EVERY == 0 and (t + 7) < T:
                    gl = gpool.tile([P, DL], mybir.dt.uint32, tag="gl")
                    gr = gpool.tile([P, DR], mybir.dt.uint32, tag="gr")
                    nc.sync.dma_start(out=gl[1:P, :],
                                      in_=X[0:P - 1, xo + C:xo + C + DL])
                    nc.sync.dma_start(out=gr[0:P - 1, :],
                                      in_=X[1:P, xo + DL:xo + DL + DR])
                    pending = (gl, gr)
                if (t + 1) % REFRESH_EVERY == 0 and (t + 1) < T:
                    gl, gr = pending
                    nc.vector.tensor_copy(X[:, xo:xo + DL], gl[:, :])
                    nc.vector.tensor_copy(X[:, xo + C + DL:xo + W], gr[:, :])
                # chunked history DMA (contiguous per partition) on the idle
                # GpSimd SWDGE queue; nothing ever waits on these except the
                # kernel end.
                if done + CHUNKS[nxt] == t + 1:
                    lo, hi = done * W, (t + 1) * W
                    nc.gpsimd.dma_start(out=hist[:, lo:hi], in_=H[:, lo:hi])
                    done = t + 1
                    nxt += 1
    nc.finalize()
    return nc


# ------------------------------------------------------------- entry points


def _run(inputs, trace=False, trace_kwargs=None):
    from concourse.bass_utils import run_bass_kernel_spmd

    t_steps = np.asarray(inputs["t_steps"], f32)
    x_grid = np.asarray(inputs["x_grid"], f32)
    initial_I = np.asarray(inputs["initial_I"], f32)
    a, b_all, c1_all = _host_params(
        t_steps, x_grid,
        np.asarray(inputs["grid1"], f32), np.asarray(inputs["spline_w1"], f32),
        np.asarray(inputs["base_w1"], f32),
        np.asarray(inputs["grid2"], f32), np.asarray(inputs["spline_w2"], f32),
        np.asarray(inputs["base_w2"], f32), np.asarray(inputs["diff_param"], f32))

    G = np.pad(initial_I, (PAD_L, PAD_R), mode="symmetric")
    J = np.rint(G.astype(f64) * S32).astype(np.uint32)
    sw = np.lib.stride_tricks.sliding_window_view(J, W)
    row0 = np.arange(P) * C
    in_maps = []
    for c in range(NCORES):
        tile = np.ascontiguousarray(sw[c * OUT + row0], dtype=np.uint32)
        in_maps.append({"x0": tile})

    nc = _build_program(a, b_all, c1_all)
    res = run_bass_kernel_spmd(nc, in_maps, core_ids=list(range(NCORES)),
                               trace=trace, trace_kwargs=trace_kwargs or {})

    out = np.empty((T, N), f32)
    inv = f32(1.0) / f32(S16)
    for c in range(NCORES):
        h = np.asarray(res.results[c]["hist"]).reshape(P, T, W)[:, :, DL:DL + C]
        flat = h.transpose(1, 0, 2).reshape(T, CORE_SLICE)
        out[:, c * OUT:(c + 1) * OUT] = (
            flat[:, HALO:HALO + OUT].astype(f32) * inv)
    return out, res


def kernel(t_steps, x_grid, initial_I, grid1, spline_w1, base_w1,
           grid2, spline_w2, base_w2, diff_param):
    out, _ = _run(dict(
        t_steps=t_steps, x_grid=x_grid, initial_I=initial_I,
        grid1=grid1, spline_w1=spline_w1, base_w1=base_w1,
        grid2=grid2, spline_w2=spline_w2, base_w2=base_w2,
        diff_param=diff_param))
    return out
